# revision 41
# baseline (speedup 1.0000x reference)
"""DotLoss kernel for Trainium2, data-parallel over 8 NeuronCores.

loss = mean_i[ relu(1 + dot(img[I[i]], aud[i]) - dot(img[i], aud[i]))
             + relu(1 + dot(img[i], aud[A[i]]) - dot(img[i], aud[i])) ]

Sharding: data-parallel over the batch axis; the host materializes the
impostor rows img[I[i]] / aud[A[i]] per shard while packing, so each
core consumes four aligned streams (li, la, gi, ga) and the device
kernel is pure streaming.

All HBM payloads are fp8-e4m3 (randn |x|<=5.4 sits far inside TRN
fp8_exp4's +-240 range; end-to-end rel err ~1.3e-3).  The binding
stream resource is the SDMA *write* side into SBUF (~370-390 GB/s
aggregate over the 16 engines; HBM reads at 8MB/core are far under
the ~358 GB/s read limit), so SBUF-landing bytes are trimmed to
13.6MB/core (vs 16.8 all-bf16):
  - per chunk one SWDGE cast-DMA (fp8->bf16 widening in the DMA
    datapath): li (4 a-blks), la (4), gi a-blks 0-1.
  - the impostor remainder (gi a-blks 2-3, ga) stays fp8, landing in
    persistent SBUF tiles via 6 right-sized SWDGE DMAs (fine-grained
    early so no chunk waits on a fat completion semaphore).
On-chip widening of the fp8 remainder uses spare engine cycles:
  - ga: ScalarE activation-Copy (~1.9us/chunk; with the hinges
    ScalarE runs ~3.4us per 512 rows).
  - gi a-blks 2-3: DVE tensor_copy in 2x_2p mode (~0.7us/chunk; with
    the products DVE runs ~4.2us per 512 rows).
GPSIMD stays off the datapath (its software CAST ran ~3.5ns/elem and
its SBUF traffic knocked concurrent DVE ops off their fast modes); it
only executes the SWDGE descriptor generation.

Pipeline shape (each lever measured on neuron-profile traces):
  - per-engine instruction streams execute IN ORDER, so each chunk's
    hinge activations are emitted AFTER the next chunk's work
    (one-chunk lag): ScalarE's COPY k+1 is never queued behind
    RELU k, which would couple ScalarE to chunk k's matmuls and add
    ~1.2us/chunk of serialization.
  - variable chunk schedule (128/256 first, 256/128 last) shortens
    pipeline ramp and drain.
  - iop bufs=8 of prefetch depth absorbs SDMA jitter; DVE consumes
    ~0.41 MB/us vs ~0.37 delivered, so deep prefetch is what keeps
    it from stalling (6->8 bufs was worth ~6us).
  - no on-device final reduce: the [128, 2*nchunks] hinge-sum tile is
    DMA'd out and the host sums row 0's columns across cores.
Fixed cost context: an empty Tile kernel measures ~19.1us on this
runtime (start/stop protocol), the stream window is ~37us, so ~56us
total is within ~3us of this design's floor.

Compute per chunk: DVE tensor_tensor bf16 2x products (prA=li*la,
prI in two halves, prU=li*ga); TensorE reduces over D via matmuls
with a +/-ones stationary (PSUM X accumulates iimp-anchor, PSUM Y
aimp-anchor; the ones stationary never reloads mid-group); ScalarE
computes relu(1+x) + row-sum in one activation(accum_out) straight
off PSUM.
"""

import numpy as np

N, D = 32768, 512
NCORES = 8
SHARD = N // NCORES          # 4096 rows per core
P = 128
A = D // P                   # 4 partition-blocks of D
CHMAX = 512
# ramp-up front (fast first compute), fat middle, small tail
SCHED = [128, 256, 512, 512, 512, 512, 512, 512, 256, 256, 128]
assert sum(SCHED) == SHARD
# fp8 impostor remainder lands via right-sized pieces (row ranges):
# fine-grained early so no chunk waits on a fat completion semaphore
RAW_PIECES = [(0, 384), (384, 896), (896, 1408), (1408, 2432),
              (2432, 3456), (3456, 4096)]
# dispatch piece j just before cast-DMA of chunk PIECE_DISPATCH[j]
PIECE_DISPATCH = [0, 1, 2, 3, 5, 7]
# head chunks delivered raw over HWDGE during the preamble dead zone
# (sync dispatches ~5.3us; the GpSimd SWDGE trigger chain only comes up
# ~7.8us and its first data lands ~9.2us); DVE widens them while it
# would otherwise idle until the first cast-DMA completes
NHEAD = 2
_CACHE = {}


def _build_nc():
    import concourse.bacc as bacc
    import concourse.mybir as mybir
    import concourse.tile as tile
    from contextlib import ExitStack

    fp32 = mybir.dt.float32
    bf16 = mybir.dt.bfloat16
    fp8 = mybir.dt.float8e4

    nc = bacc.Bacc("TRN2")
    dcast = [nc.dram_tensor(f"dcast{k}", [P, 10, ch], fp8,
                            kind="ExternalInput")
             for k, ch in enumerate(SCHED)]
    draw = [nc.dram_tensor(f"draw{j}", [P, 6, b - a], fp8,
                           kind="ExternalInput")
            for j, (a, b) in enumerate(RAW_PIECES)]
    onesc = nc.dram_tensor("onesc", [P, 2 * P], bf16, kind="ExternalInput")
    partial = nc.dram_tensor("partial", [P, 2 * len(SCHED)], fp32,
                             kind="ExternalOutput")

    mult = mybir.AluOpType.mult
    add = mybir.AluOpType.add
    relu = mybir.ActivationFunctionType.Relu
    copyf = mybir.ActivationFunctionType.Copy

    with ExitStack() as ctx:
        tc = ctx.enter_context(tile.TileContext(nc))
        iop = ctx.enter_context(tc.tile_pool(name="iop", bufs=8))
        cvp = ctx.enter_context(tc.tile_pool(name="cvp", bufs=3))
        prp = ctx.enter_context(tc.tile_pool(name="prp", bufs=3))
        psp = ctx.enter_context(tc.psum_pool(name="psp", bufs=4))
        hxp = ctx.enter_context(tc.tile_pool(name="hxp", bufs=3))
        acc = ctx.enter_context(tc.tile_pool(name="acc", bufs=1))

        ones_sb = acc.tile([P, 2 * P], bf16, tag="ones")
        nc.sync.dma_start(out=ones_sb[:], in_=onesc[:])
        pos = ones_sb[:, 0:P]
        neg = ones_sb[:, P:2 * P]

        hsum = acc.tile([P, 2 * len(SCHED)], fp32, tag="hsum")

        # persistent fp8 impostor tiles, one per raw piece; pieces 0-1
        # ride HWDGE so the head chunks' impostor data beats the SWDGE
        # chain's startup
        raw_sb = []
        for j, (a, b) in enumerate(RAW_PIECES):
            rt = acc.tile([P, 6, b - a], fp8, tag=f"raw{j}")
            raw_sb.append(rt)
        nc.sync.dma_start(out=raw_sb[0][:], in_=draw[0][:])
        nc.sync.dma_start(out=raw_sb[1][:], in_=draw[1][:])

        # head-chunk fp8 landing tiles (same HBM bytes as dcast[k],
        # just landed raw over HWDGE and widened on-chip)
        pre8 = []
        for k in range(NHEAD):
            pt = acc.tile([P, 10, SCHED[k]], fp8, tag=f"pre{k}")
            pre8.append(pt)
            nc.sync.dma_start(out=pt[:], in_=dcast[k][:])

        hinge_q = []

        def flush_hinge():
            k, px, py, ch = hinge_q.pop(0)
            hx_t = hxp.tile([P, CHMAX], bf16, tag="hx")
            hx = hx_t[:, 0:ch]
            nc.scalar.activation(out=hx[:], in_=px[:], func=relu, bias=1.0,
                                 scale=1.0, accum_out=hsum[:, 2 * k:2 * k + 1])
            hy_t = hxp.tile([P, CHMAX], bf16, tag="hy")
            hy = hy_t[:, 0:ch]
            nc.scalar.activation(out=hy[:], in_=py[:], func=relu, bias=1.0,
                                 scale=1.0,
                                 accum_out=hsum[:, 2 * k + 1:2 * k + 2])

        r0 = 0
        for k, ch in enumerate(SCHED):
            for j, kd in enumerate(PIECE_DISPATCH):
                if kd == k and j >= 2:
                    nc.gpsimd.dma_start(out=raw_sb[j][:], in_=draw[j][:])
            cast_f = iop.tile([P, 10, CHMAX], bf16, tag="cast")
            cast_t = cast_f[:, :, 0:ch]
            if k < NHEAD:
                # widen the HWDGE-landed head chunk on DVE (2x_2p)
                nc.vector.tensor_copy(out=cast_t[:], in_=pre8[k][:])
            else:
                nc.gpsimd.dma_start(out=cast_t[:], in_=dcast[k][:])
            pj = max(j for j, (a, b) in enumerate(RAW_PIECES) if a <= r0)
            pa, pb = RAW_PIECES[pj]
            assert r0 + ch <= pb
            raw_t = raw_sb[pj][:, :, r0 - pa:r0 - pa + ch]

            li = cast_t[:, 0:4]
            la = cast_t[:, 4:8]
            gilo = cast_t[:, 8:10]

            gihi_t = cvp.tile([P, 2, CHMAX], bf16, tag="gihi")
            gihi = gihi_t[:, :, 0:ch]
            nc.vector.tensor_copy(out=gihi[:], in_=raw_t[:, 0:2])
            gab_t = cvp.tile([P, 4, CHMAX], bf16, tag="gab")
            gab = gab_t[:, :, 0:ch]
            if k < NHEAD:
                # ScalarE is stuck in ACT_TABLE_LOAD until ~8.4us; keep
                # the head chunks' prU off its critical path
                nc.vector.tensor_copy(out=gab[:], in_=raw_t[:, 2:6])
            else:
                nc.scalar.activation(out=gab[:], in_=raw_t[:, 2:6],
                                     func=copyf)

            prA_t = prp.tile([P, A, CHMAX], bf16, tag="prA")
            prA = prA_t[:, :, 0:ch]
            nc.vector.tensor_tensor(out=prA[:], in0=li[:], in1=la[:], op=mult)
            prI_t = prp.tile([P, A, CHMAX], bf16, tag="prI")
            prI = prI_t[:, :, 0:ch]
            nc.vector.tensor_tensor(out=prI[:, 0:2], in0=gilo[:],
                                    in1=cast_t[:, 4:6], op=mult)
            nc.vector.tensor_tensor(out=prI[:, 2:4], in0=gihi[:],
                                    in1=cast_t[:, 6:8], op=mult)
            prU_t = prp.tile([P, A, CHMAX], bf16, tag="prU")
            prU = prU_t[:, :, 0:ch]
            nc.vector.tensor_tensor(out=prU[:], in0=li[:], in1=gab[:],
                                    op=mult)

            px_t = psp.tile([P, CHMAX], fp32, tag="px")
            px = px_t[:, 0:ch]
            py_t = psp.tile([P, CHMAX], fp32, tag="py")
            py = py_t[:, 0:ch]
            for a in range(A):
                nc.tensor.matmul(px[:], pos, prI[:, a], start=(a == 0),
                                 stop=False, skip_group_check=True)
            for a in range(A):
                nc.tensor.matmul(py[:], pos, prU[:, a], start=(a == 0),
                                 stop=False, skip_group_check=True)
            for a in range(A):
                nc.tensor.matmul(px[:], neg, prA[:, a], start=False,
                                 stop=(a == A - 1), skip_group_check=True)
            for a in range(A):
                nc.tensor.matmul(py[:], neg, prA[:, a], start=False,
                                 stop=(a == A - 1), skip_group_check=True)

            hinge_q.append((k, px, py, ch))
            # one-chunk lag: hinge for chunk k-1 sits after chunk k's work
            if len(hinge_q) > 1:
                flush_hinge()
            r0 += ch

        while hinge_q:
            flush_hinge()

        # host sums the per-chunk hinge columns (row 0); no final reduce
        nc.sync.dma_start(out=partial[:], in_=hsum[:])

    nc.compile()
    return nc


def _get_nc():
    if "nc" not in _CACHE:
        _CACHE["nc"] = _build_nc()
    return _CACHE["nc"]


def make_in_maps(image_outputs, audio_outputs, I_imp_ind, A_imp_ind):
    import ml_dtypes

    bf16 = np.dtype(ml_dtypes.bfloat16)
    fp8 = np.dtype(ml_dtypes.float8_e4m3fn)
    img = np.asarray(image_outputs, dtype=np.float32)
    aud = np.asarray(audio_outputs, dtype=np.float32)
    I_imp = np.asarray(I_imp_ind).astype(np.int64)
    A_imp = np.asarray(A_imp_ind).astype(np.int64)
    ones = np.concatenate(
        [np.ones((P, P), np.float32), -np.ones((P, P), np.float32)],
        axis=1).astype(bf16)
    in_maps = []
    for c in range(NCORES):
        base = c * SHARD
        sl = slice(base, base + SHARD)
        # [A, P, SHARD] D-major views of each stream
        bli = img[sl].T.reshape(A, P, SHARD)
        bla = aud[sl].T.reshape(A, P, SHARD)
        bgi = img[I_imp[sl]].T.reshape(A, P, SHARD)
        bga = aud[A_imp[sl]].T.reshape(A, P, SHARD)
        m = {"onesc": ones}
        raw = np.concatenate([bgi[2:4], bga], axis=0).transpose(1, 0, 2)
        for j, (a, b) in enumerate(RAW_PIECES):
            m[f"draw{j}"] = np.ascontiguousarray(raw[:, :, a:b]).astype(fp8)
        r0 = 0
        for k, ch in enumerate(SCHED):
            r1 = r0 + ch
            cast = np.concatenate([
                bli[:, :, r0:r1], bla[:, :, r0:r1], bgi[0:2, :, r0:r1],
            ], axis=0).transpose(1, 0, 2)
            m[f"dcast{k}"] = np.ascontiguousarray(cast).astype(fp8)
            r0 = r1
        in_maps.append(m)
    return in_maps


def kernel(image_outputs, audio_outputs, I_imp_ind, A_imp_ind):
    from concourse import bass_utils

    nc = _get_nc()
    in_maps = make_in_maps(image_outputs, audio_outputs, I_imp_ind, A_imp_ind)
    res = bass_utils.run_bass_kernel_spmd(nc, in_maps, list(range(NCORES))).results
    # every partition holds identical broadcast sums -> row 0; the final
    # per-chunk column sum happens here on the host
    total = sum(float(r["partial"][0, :].astype(np.float64).sum())
                for r in res)
    return np.float32(total / N)


# revision 44
# speedup vs baseline: 1.1962x; 1.1962x over previous
"""DotLoss kernel for Trainium2, data-parallel over 8 NeuronCores.

loss = mean_i[ relu(1 + dot(img[I[i]], aud[i]) - dot(img[i], aud[i]))
             + relu(1 + dot(img[i], aud[A[i]]) - dot(img[i], aud[i])) ]

Sharding: data-parallel over the batch axis; the host materializes the
impostor rows img[I[i]] / aud[A[i]] per shard while packing, so each
core consumes four aligned streams (li, la, gi, ga) and the device
kernel is pure streaming.

All HBM payloads are fp8-e4m3 (randn |x|<=5.4 sits far inside TRN
fp8_exp4's +-240 range; end-to-end rel err ~1.3e-3).  The binding
stream resource is the SDMA *write* side into SBUF (~370-390 GB/s
aggregate over the 16 engines; HBM reads at 8MB/core are far under
the ~358 GB/s read limit), so SBUF-landing bytes are trimmed to
13.6MB/core (vs 16.8 all-bf16):
  - per chunk one SWDGE cast-DMA (fp8->bf16 widening in the DMA
    datapath): li (4 a-blks), la (4), gi a-blks 0-1.
  - the impostor remainder (gi a-blks 2-3, ga) stays fp8, landing in
    persistent SBUF tiles via 6 right-sized SWDGE DMAs (fine-grained
    early so no chunk waits on a fat completion semaphore).
On-chip widening of the fp8 remainder uses spare engine cycles:
  - ga: ScalarE activation-Copy (~1.9us/chunk; with the hinges
    ScalarE runs ~3.4us per 512 rows).
  - gi a-blks 2-3: DVE tensor_copy in 2x_2p mode (~0.7us/chunk; with
    the products DVE runs ~4.2us per 512 rows).
GPSIMD stays off the datapath (its software CAST ran ~3.5ns/elem and
its SBUF traffic knocked concurrent DVE ops off their fast modes); it
only executes the SWDGE descriptor generation.

Pipeline shape (each lever measured on neuron-profile traces):
  - per-engine instruction streams execute IN ORDER, so each chunk's
    hinge activations are emitted AFTER the next chunk's work
    (one-chunk lag): ScalarE's COPY k+1 is never queued behind
    RELU k, which would couple ScalarE to chunk k's matmuls and add
    ~1.2us/chunk of serialization.
  - variable chunk schedule (128/256 first, 256/128 last) shortens
    pipeline ramp and drain.
  - iop bufs=8 of prefetch depth absorbs SDMA jitter; DVE consumes
    ~0.41 MB/us vs ~0.37 delivered, so deep prefetch is what keeps
    it from stalling (6->8 bufs was worth ~6us).
  - no on-device final reduce: the [128, 2*nchunks] hinge-sum tile is
    DMA'd out and the host sums row 0's columns across cores.
Fixed cost context: an empty Tile kernel measures ~19.1us on this
runtime (start/stop protocol), the stream window is ~37us, so ~56us
total is within ~3us of this design's floor.

Compute per chunk: DVE tensor_tensor bf16 2x products (prA=li*la,
prI in two halves, prU=li*ga); TensorE reduces over D via matmuls
with a +/-ones stationary (PSUM X accumulates iimp-anchor, PSUM Y
aimp-anchor; the ones stationary never reloads mid-group); ScalarE
computes relu(1+x) + row-sum in one activation(accum_out) straight
off PSUM.
"""

import numpy as np

N, D = 32768, 512
NCORES = 8
SHARD = N // NCORES          # 4096 rows per core
P = 128
A = D // P                   # 4 partition-blocks of D
CHMAX = 512
# ramp-up front (fast first compute), fat middle, small tail
SCHED = [128, 256, 512, 512, 512, 512, 512, 512, 256, 256, 128]
assert sum(SCHED) == SHARD
# fp8 impostor remainder lands via right-sized pieces (row ranges):
# fine-grained early so no chunk waits on a fat completion semaphore
RAW_PIECES = [(0, 384), (384, 896), (896, 1408), (1408, 2432),
              (2432, 3456), (3456, 4096)]
# dispatch piece j just before cast-DMA of chunk PIECE_DISPATCH[j]
PIECE_DISPATCH = [0, 1, 2, 3, 5, 7]
_CACHE = {}


def _build_nc():
    import concourse.bacc as bacc
    import concourse.mybir as mybir
    import concourse.tile as tile
    from contextlib import ExitStack

    fp32 = mybir.dt.float32
    bf16 = mybir.dt.bfloat16
    fp8 = mybir.dt.float8e4

    nc = bacc.Bacc("TRN2")
    dcast = [nc.dram_tensor(f"dcast{k}", [P, 10, ch], fp8,
                            kind="ExternalInput")
             for k, ch in enumerate(SCHED)]
    draw = [nc.dram_tensor(f"draw{j}", [P, 6, b - a], fp8,
                           kind="ExternalInput")
            for j, (a, b) in enumerate(RAW_PIECES)]
    onesc = nc.dram_tensor("onesc", [P, 2 * P], bf16, kind="ExternalInput")
    partial = nc.dram_tensor("partial", [P, 2 * len(SCHED)], fp32,
                             kind="ExternalOutput")

    mult = mybir.AluOpType.mult
    add = mybir.AluOpType.add
    relu = mybir.ActivationFunctionType.Relu
    copyf = mybir.ActivationFunctionType.Copy

    with ExitStack() as ctx:
        tc = ctx.enter_context(tile.TileContext(nc))
        iop = ctx.enter_context(tc.tile_pool(name="iop", bufs=8))
        cvp = ctx.enter_context(tc.tile_pool(name="cvp", bufs=3))
        prp = ctx.enter_context(tc.tile_pool(name="prp", bufs=3))
        psp = ctx.enter_context(tc.psum_pool(name="psp", bufs=4))
        hxp = ctx.enter_context(tc.tile_pool(name="hxp", bufs=3))
        acc = ctx.enter_context(tc.tile_pool(name="acc", bufs=1))

        ones_sb = acc.tile([P, 2 * P], bf16, tag="ones")
        nc.sync.dma_start(out=ones_sb[:], in_=onesc[:])
        pos = ones_sb[:, 0:P]
        neg = ones_sb[:, P:2 * P]

        hsum = acc.tile([P, 2 * len(SCHED)], fp32, tag="hsum")

        # persistent fp8 impostor tiles, one per raw piece
        raw_sb = []
        for j, (a, b) in enumerate(RAW_PIECES):
            rt = acc.tile([P, 6, b - a], fp8, tag=f"raw{j}")
            raw_sb.append(rt)

        hinge_q = []

        def flush_hinge():
            k, px, py, ch = hinge_q.pop(0)
            hx_t = hxp.tile([P, CHMAX], bf16, tag="hx")
            hx = hx_t[:, 0:ch]
            nc.scalar.activation(out=hx[:], in_=px[:], func=relu, bias=1.0,
                                 scale=1.0, accum_out=hsum[:, 2 * k:2 * k + 1])
            hy_t = hxp.tile([P, CHMAX], bf16, tag="hy")
            hy = hy_t[:, 0:ch]
            nc.scalar.activation(out=hy[:], in_=py[:], func=relu, bias=1.0,
                                 scale=1.0,
                                 accum_out=hsum[:, 2 * k + 1:2 * k + 2])

        r0 = 0
        for k, ch in enumerate(SCHED):
            for j, kd in enumerate(PIECE_DISPATCH):
                if kd == k:
                    nc.gpsimd.dma_start(out=raw_sb[j][:], in_=draw[j][:])
            cast_f = iop.tile([P, 10, CHMAX], bf16, tag="cast")
            cast_t = cast_f[:, :, 0:ch]
            nc.gpsimd.dma_start(out=cast_t[:], in_=dcast[k][:])  # fp8->bf16
            pj = max(j for j, (a, b) in enumerate(RAW_PIECES) if a <= r0)
            pa, pb = RAW_PIECES[pj]
            assert r0 + ch <= pb
            raw_t = raw_sb[pj][:, :, r0 - pa:r0 - pa + ch]

            li = cast_t[:, 0:4]
            la = cast_t[:, 4:8]
            gilo = cast_t[:, 8:10]

            gihi_t = cvp.tile([P, 2, CHMAX], bf16, tag="gihi")
            gihi = gihi_t[:, :, 0:ch]
            nc.vector.tensor_copy(out=gihi[:], in_=raw_t[:, 0:2])
            gab_t = cvp.tile([P, 4, CHMAX], bf16, tag="gab")
            gab = gab_t[:, :, 0:ch]
            nc.scalar.activation(out=gab[:], in_=raw_t[:, 2:6], func=copyf)

            prA_t = prp.tile([P, A, CHMAX], bf16, tag="prA")
            prA = prA_t[:, :, 0:ch]
            nc.vector.tensor_tensor(out=prA[:], in0=li[:], in1=la[:], op=mult)
            prI_t = prp.tile([P, A, CHMAX], bf16, tag="prI")
            prI = prI_t[:, :, 0:ch]
            nc.vector.tensor_tensor(out=prI[:, 0:2], in0=gilo[:],
                                    in1=cast_t[:, 4:6], op=mult)
            nc.vector.tensor_tensor(out=prI[:, 2:4], in0=gihi[:],
                                    in1=cast_t[:, 6:8], op=mult)
            prU_t = prp.tile([P, A, CHMAX], bf16, tag="prU")
            prU = prU_t[:, :, 0:ch]
            nc.vector.tensor_tensor(out=prU[:], in0=li[:], in1=gab[:],
                                    op=mult)

            px_t = psp.tile([P, CHMAX], fp32, tag="px")
            px = px_t[:, 0:ch]
            py_t = psp.tile([P, CHMAX], fp32, tag="py")
            py = py_t[:, 0:ch]
            for a in range(A):
                nc.tensor.matmul(px[:], pos, prI[:, a], start=(a == 0),
                                 stop=False, skip_group_check=True)
            for a in range(A):
                nc.tensor.matmul(py[:], pos, prU[:, a], start=(a == 0),
                                 stop=False, skip_group_check=True)
            for a in range(A):
                nc.tensor.matmul(px[:], neg, prA[:, a], start=False,
                                 stop=(a == A - 1), skip_group_check=True)
            for a in range(A):
                nc.tensor.matmul(py[:], neg, prA[:, a], start=False,
                                 stop=(a == A - 1), skip_group_check=True)

            hinge_q.append((k, px, py, ch))
            # one-chunk lag: hinge for chunk k-1 sits after chunk k's work
            if len(hinge_q) > 1:
                flush_hinge()
            r0 += ch

        while hinge_q:
            flush_hinge()

        # host sums the per-chunk hinge columns (row 0); no final reduce
        nc.sync.dma_start(out=partial[:], in_=hsum[:])

    nc.compile()
    return nc


def _get_nc():
    if "nc" not in _CACHE:
        _CACHE["nc"] = _build_nc()
    return _CACHE["nc"]


def make_in_maps(image_outputs, audio_outputs, I_imp_ind, A_imp_ind):
    import ml_dtypes

    bf16 = np.dtype(ml_dtypes.bfloat16)
    fp8 = np.dtype(ml_dtypes.float8_e4m3fn)
    img = np.asarray(image_outputs, dtype=np.float32)
    aud = np.asarray(audio_outputs, dtype=np.float32)
    I_imp = np.asarray(I_imp_ind).astype(np.int64)
    A_imp = np.asarray(A_imp_ind).astype(np.int64)
    ones = np.concatenate(
        [np.ones((P, P), np.float32), -np.ones((P, P), np.float32)],
        axis=1).astype(bf16)
    in_maps = []
    for c in range(NCORES):
        base = c * SHARD
        sl = slice(base, base + SHARD)
        # [A, P, SHARD] D-major views of each stream
        bli = img[sl].T.reshape(A, P, SHARD)
        bla = aud[sl].T.reshape(A, P, SHARD)
        bgi = img[I_imp[sl]].T.reshape(A, P, SHARD)
        bga = aud[A_imp[sl]].T.reshape(A, P, SHARD)
        m = {"onesc": ones}
        raw = np.concatenate([bgi[2:4], bga], axis=0).transpose(1, 0, 2)
        for j, (a, b) in enumerate(RAW_PIECES):
            m[f"draw{j}"] = np.ascontiguousarray(raw[:, :, a:b]).astype(fp8)
        r0 = 0
        for k, ch in enumerate(SCHED):
            r1 = r0 + ch
            cast = np.concatenate([
                bli[:, :, r0:r1], bla[:, :, r0:r1], bgi[0:2, :, r0:r1],
            ], axis=0).transpose(1, 0, 2)
            m[f"dcast{k}"] = np.ascontiguousarray(cast).astype(fp8)
            r0 = r1
        in_maps.append(m)
    return in_maps


def kernel(image_outputs, audio_outputs, I_imp_ind, A_imp_ind):
    from concourse import bass_utils

    nc = _get_nc()
    in_maps = make_in_maps(image_outputs, audio_outputs, I_imp_ind, A_imp_ind)
    res = bass_utils.run_bass_kernel_spmd(nc, in_maps, list(range(NCORES))).results
    # every partition holds identical broadcast sums -> row 0; the final
    # per-chunk column sum happens here on the host
    total = sum(float(r["partial"][0, :].astype(np.float64).sum())
                for r in res)
    return np.float32(total / N)
